# revision 8
# baseline (speedup 1.0000x reference)
"""Conditional (per-row expert) linear layer for Trainium2, 8 NeuronCores.

Math: out[i] = W[c_i] @ x[i] + sum_c b[c]    (x: [B,D], W: [C,D,D], b: [C,D])

Strategy: expert-parallel. Core c handles exactly the rows with
condition_ids == c (gathered on host, padded to a common capacity so the
SPMD NEFF has static shapes). Each core runs one [n_cap, D] @ [D, D] GEMM
with bf16 operands (fp32 PSUM accumulation), the host scatters rows back
and adds the (row-independent) summed bias in fp32. bf16 halves HBM
traffic vs fp32 (x 2.25MB + W 2MB + out 2.25MB per core) and enables the
PE's fast-weight-load path, so the kernel is bound by PE streaming:
n_cap*D*D/128^2 cycles @ 2.4GHz (~30.7us at n_cap=1152) plus per-k-tile
weight loads.
"""

import sys
from contextlib import ExitStack

import numpy as np
import ml_dtypes

try:
    import concourse.bass as bass  # noqa: F401
except ImportError:  # pragma: no cover
    sys.path.insert(0, "/opt/trn_rl_repo")

import jax
from jax.experimental.shard_map import shard_map
from jax.sharding import Mesh, PartitionSpec

import concourse.mybir as mybir
import concourse.tile as tile
from concourse import bacc
from concourse import bass2jax as _b2j

B, D, C = 8192, 1024, 8
P = 128  # partitions
KT = D // P  # k-tiles along the contraction dim
HALF = 512  # PSUM half-bank free size (fp32)
BF16 = ml_dtypes.bfloat16

_cache: dict[tuple, "_Runner"] = {}


def _row_tiles(n_cap: int):
    """Row tiles sized so every tile is >64 rows: the matmul tile_size
    column entry then rounds to 128 for all tiles, keeping the PE in one
    uniform 64x128 tiling mode (mode switches drain the array)."""
    n_full = n_cap // P
    rem = n_cap - n_full * P
    sizes = [P] * n_full
    if rem:
        if rem <= 64 and n_full:
            sizes = sizes[:-1] + [(P + rem) // 2, P + rem - (P + rem) // 2]
        else:
            sizes.append(rem)
    tiles, s = [], 0
    for sz in sizes:
        tiles.append((s, sz))
        s += sz
    assert s == n_cap
    return tiles


def _build(n_cap: int, reps: int = 1):
    """Per-core program: out[n, o] = xT.T @ WT, n_cap x D output, bf16 io.

    The contraction is split into K=64 sub-tiles living in alternating
    row-groups of the PE array (partitions 0:64 / 64:128 of each k-slab).
    The PE's reorder window pulls each LDWEIGHTS ahead of the other row
    group's in-flight matmul, and the two row-groups' matmuls stream
    concurrently, so the per-matmul weight-load tax is hidden. Even/odd
    sub-tiles accumulate into separate PSUM banks (concurrent access to
    one bank from different row tiles is illegal) and are summed during
    PSUM evacuation (ACT copy + DVE add). The output is processed in two
    512-column passes so a row-tile only holds 2 PSUM banks, giving the
    scheduler ~4 row-tiles of lookahead to ride out the input DMA fill.

    reps > 1 repeats the whole body (including all DMAs) back-to-back for
    benchmarking: wall(T) - wall(1) isolates per-execution device time."""
    assert n_cap % 32 == 0
    row_tiles = _row_tiles(n_cap)
    nc = bacc.Bacc("TRN2", target_bir_lowering=False, debug=False, num_devices=8, num_swdge_queues=4)
    xT = nc.dram_tensor("xT", [D, n_cap], mybir.dt.bfloat16, kind="ExternalInput").ap()
    WT = nc.dram_tensor("WT", [D, D], mybir.dt.bfloat16, kind="ExternalInput").ap()
    out = nc.dram_tensor("out", [n_cap, D], mybir.dt.bfloat16, kind="ExternalOutput").ap()

    with tile.TileContext(nc) as tc, ExitStack() as ctx:
        w_pool = ctx.enter_context(tc.tile_pool(name="w", bufs=2))
        x_pool = ctx.enter_context(tc.tile_pool(name="x", bufs=2))
        o_pool = ctx.enter_context(tc.tile_pool(name="o", bufs=4))
        ps_pool = ctx.enter_context(tc.tile_pool(name="ps", bufs=4, space="PSUM"))

        xh = n_cap // 64 * 32  # x column split point (row dim), 32-aligned

        for _rep in range(reps):
            w_tiles, x_tiles = [], []
            for k in range(KT):
                w_tiles.append(
                    w_pool.tile([P, D], mybir.dt.bfloat16, name=f"wt{k}", tag=f"wt{k}")
                )
                x_tiles.append(
                    x_pool.tile([P, n_cap], mybir.dt.bfloat16, name=f"xt{k}", tag=f"xt{k}")
                )
            # W lands h0-halves first to match the pass order below.
            for lo in (0, HALF):
                for k in range(KT):
                    nc.sync.dma_start(
                        w_tiles[k][:, lo : lo + HALF], WT[k * P : (k + 1) * P, lo : lo + HALF]
                    )
            for k in range(KT):
                nc.gpsimd.dma_start(x_tiles[k][:, 0:xh], xT[k * P : (k + 1) * P, 0:xh])
                nc.gpsimd.dma_start(x_tiles[k][:, xh:n_cap], xT[k * P : (k + 1) * P, xh:n_cap])

            for lo in (0, HALF):
                for ti, (start, size) in enumerate(row_tiles):
                    psE = ps_pool.tile([P, HALF], mybir.dt.float32, name="psE", tag="psE")
                    psO = ps_pool.tile([P, HALF], mybir.dt.float32, name="psO", tag="psO")
                    for k in range(KT):
                        for kp, ps in ((0, psE), (64, psO)):
                            nc.tensor.matmul(
                                ps[:size, :],
                                x_tiles[k][kp : kp + 64, start : start + size],
                                w_tiles[k][kp : kp + 64, lo : lo + HALF],
                                start=(k == 0),
                                stop=(k == KT - 1),
                                skip_group_check=True,
                            )
                    o_sb = o_pool.tile(
                        [P, HALF], mybir.dt.float32, name=f"o{lo}_{start}", tag="o_sb"
                    )
                    nc.scalar.copy(o_sb[:size, :], psE[:size, :])
                    ob = o_pool.tile(
                        [P, HALF], mybir.dt.bfloat16, name=f"ob{lo}_{start}", tag="ob"
                    )
                    nc.vector.tensor_add(ob[:size, :], o_sb[:size, :], psO[:size, :])
                    out_eng = nc.scalar if ti % 2 == 0 else nc.gpsimd
                    out_eng.dma_start(out[start : start + size, lo : lo + HALF], ob[:size, :])

    nc.compile()
    _check_noload_pairs(nc)
    return nc


def _check_noload_pairs(nc):
    """Every ldweights=False matmul must execute (in PE stream order) with
    the stationary operand most recently loaded into its row group — by a
    standalone InstLdweights or a self-loading matmul. Scheduling is
    deterministic at build time, so passing here guarantees correctness on
    device."""
    loaded: dict = {}
    for fn in nc.m.functions:
        for blk in fn.blocks:
            for inst in blk.instructions:
                tn = type(inst).__name__
                if tn == "InstLdweights":
                    rg = (inst.tile_position or (0, 0))[0]
                    loaded[rg] = str(inst.ins[0])
                elif tn == "InstMatmult":
                    rg = (inst.tile_position or (0, 0))[0]
                    if inst.ldweights is False:
                        assert loaded.get(rg) == str(inst.ins[1]), (
                            f"no-load matmul stationary mismatch in rg {rg}:\n"
                            f"loaded: {loaded.get(rg)}\nthis: {inst.ins[1]}"
                        )
                    else:
                        loaded[rg] = str(inst.ins[1])


class _Runner:
    """Caches the compiled NEFF + jitted shard_map executable for one n_cap."""

    def __init__(self, n_cap: int, reps: int = 1):
        self.n_cap = n_cap
        self.nc = _build(n_cap, reps)
        _b2j.install_neuronx_cc_hook()

        assert self.nc.dbg_addr is None
        partition_name = (
            self.nc.partition_id_tensor.name if self.nc.partition_id_tensor else None
        )

        in_names, out_names, out_avals = [], [], []
        for alloc in self.nc.m.functions[0].allocations:
            if not isinstance(alloc, mybir.MemoryLocationSet):
                continue
            name = alloc.memorylocations[0].name
            if alloc.kind == "ExternalInput":
                if name != partition_name:
                    in_names.append(name)
            elif alloc.kind == "ExternalOutput":
                out_names.append(name)
                out_avals.append(
                    jax.core.ShapedArray(
                        tuple(alloc.tensor_shape), mybir.dt.np(alloc.dtype)
                    )
                )
        self.in_names = in_names
        self.out_names = out_names
        self.out_avals = out_avals
        self.n_params = len(in_names)
        self.n_outs = len(out_names)
        all_in_names = tuple(in_names + out_names)
        if partition_name is not None:
            all_in_names = all_in_names + (partition_name,)

        nc = self.nc

        def _bind(*args):
            operands = list(args)
            if partition_name is not None:
                operands.append(_b2j.partition_id_tensor())
            return tuple(
                _b2j._bass_exec_p.bind(
                    *operands,
                    out_avals=tuple(out_avals),
                    in_names=all_in_names,
                    out_names=tuple(out_names),
                    lowering_input_output_aliases=(),
                    sim_require_finite=True,
                    sim_require_nnan=True,
                    nc=nc,
                )
            )

        self._bind = _bind
        self.devices = jax.devices("neuron")[:C]
        self.mesh = Mesh(np.asarray(self.devices), ("core",))
        spec_in = (PartitionSpec("core"),) * (self.n_params + self.n_outs)
        spec_out = (PartitionSpec("core"),) * self.n_outs
        self._spec_in, self._spec_out = spec_in, spec_out
        self._exec = jax.jit(
            shard_map(
                _bind,
                mesh=self.mesh,
                in_specs=spec_in,
                out_specs=spec_out,
                check_rep=False,
            ),
            donate_argnums=tuple(range(self.n_params, self.n_params + self.n_outs)),
            keep_unused=True,
        )

    def make_exec_nodonate(self):
        """Jitted executable that does not donate its output-init operands,
        so pre-staged device args can be reused across timing reps."""
        return jax.jit(
            shard_map(
                self._bind,
                mesh=self.mesh,
                in_specs=self._spec_in,
                out_specs=self._spec_out,
                check_rep=False,
            ),
            keep_unused=True,
        )

    def concat_inputs(self, in_maps):
        return [
            np.concatenate([np.asarray(m[name]) for m in in_maps], axis=0)
            for name in self.in_names
        ]

    def zero_outs(self):
        return [
            np.zeros((C * a.shape[0], *a.shape[1:]), a.dtype) for a in self.out_avals
        ]

    def run(self, in_maps):
        out_arrs = self._exec(*self.concat_inputs(in_maps), *self.zero_outs())
        return [
            {
                name: np.asarray(out_arrs[i]).reshape(C, *self.out_avals[i].shape)[c]
                for i, name in enumerate(self.out_names)
            }
            for c in range(C)
        ]


def _get(n_cap: int, reps: int = 1) -> _Runner:
    key = (n_cap, reps)
    if key not in _cache:
        _cache[key] = _Runner(n_cap, reps)
    return _cache[key]


def _prep(x, condition_ids, W, b):
    x = np.asarray(x, dtype=np.float32)
    cond = np.asarray(condition_ids).astype(np.int64)
    W = np.asarray(W, dtype=np.float32)
    b = np.asarray(b, dtype=np.float32)

    bias_sum = b.sum(axis=0, dtype=np.float32)

    rows = [np.nonzero(cond == c)[0] for c in range(C)]
    n_max = max(len(r) for r in rows)
    n_cap = max(32, -(-n_max // 32) * 32)

    in_maps = []
    for c in range(C):
        r = rows[c]
        xg = np.zeros((n_cap, D), BF16)
        xg[: len(r)] = x[r].astype(BF16)
        in_maps.append(
            {
                "xT": np.ascontiguousarray(xg.T),
                "WT": np.ascontiguousarray(W[c].T.astype(BF16)),
            }
        )
    return rows, n_cap, in_maps, bias_sum


def _run(x, condition_ids, W, b, trace=False):
    rows, n_cap, in_maps, bias_sum = _prep(x, condition_ids, W, b)
    runner = _get(n_cap)
    results = runner.run(in_maps)

    out = np.empty((B, D), np.float32)
    for c in range(C):
        r = rows[c]
        out[r] = results[c]["out"][: len(r)].astype(np.float32) + bias_sum
    return out, runner


def kernel(x, condition_ids, W, b):
    out, _ = _run(x, condition_ids, W, b)
    return out


# revision 12
# speedup vs baseline: 1.3397x; 1.3397x over previous
"""Conditional (per-row expert) linear layer for Trainium2, 8 NeuronCores.

Math: out[i] = W[c_i] @ x[i] + sum_c b[c]    (x: [B,D], W: [C,D,D], b: [C,D])

Strategy: expert-parallel. Core c handles exactly the rows with
condition_ids == c (gathered on host, padded to a common capacity so the
SPMD NEFF has static shapes). Each core runs one [n_cap, D] @ [D, D] GEMM
with bf16 operands (fp32 PSUM accumulation), the host scatters rows back
and adds the (row-independent) summed bias in fp32. bf16 halves HBM
traffic vs fp32 (x 2.25MB + W 2MB + out 2.25MB per core) and enables the
PE's fast-weight-load path, so the kernel is bound by PE streaming:
n_cap*D*D/128^2 cycles @ 2.4GHz (~30.7us at n_cap=1152) plus per-k-tile
weight loads.
"""

import sys
from contextlib import ExitStack

import numpy as np
import ml_dtypes

try:
    import concourse.bass as bass  # noqa: F401
except ImportError:  # pragma: no cover
    sys.path.insert(0, "/opt/trn_rl_repo")

import jax
from jax.experimental.shard_map import shard_map
from jax.sharding import Mesh, PartitionSpec

import concourse.mybir as mybir
import concourse.tile as tile
from concourse import bacc
from concourse import bass2jax as _b2j

B, D, C = 8192, 1024, 8
P = 128  # partitions
KT = D // P  # k-tiles along the contraction dim
HALF = 512  # PSUM half-bank free size (fp32)
BF16 = ml_dtypes.bfloat16

_cache: dict[tuple, "_Runner"] = {}


def _chunks(n_cap: int):
    """Split the row range into near-equal chunks of <=512 (PSUM bank)."""
    n = -(-n_cap // HALF)
    base = n_cap // n
    sizes = [base + (1 if i < n_cap - base * n else 0) for i in range(n)]
    out, s = [], 0
    for sz in sizes:
        out.append((s, sz))
        s += sz
    assert s == n_cap
    return out


def _dedup_ldweights(nc):
    """The Tile scheduler pairs every InstMatmult with its own
    InstLdweights even when consecutive matmuls share the stationary
    operand. Drop the redundant reloads (PE stream order, same stationary
    AP, no sync side effects) — each costs ~53ns of serial PE time.
    Runs before nc.compile() so the wait-migration passes see the final
    stream; _check_noload_pairs re-verifies after compile."""
    removed = 0
    for fn in nc.m.functions:
        for blk in fn.blocks:
            last_ldw = None
            dead = []
            for inst in blk.instructions:
                tn = type(inst).__name__
                if tn == "InstLdweights":
                    cur = str(inst.ins[0])
                    si = inst.sync_info
                    clean = si is None or (not si.on_wait and not si.on_update)
                    if last_ldw == cur and clean:
                        dead.append(inst)
                    else:
                        last_ldw = cur
                elif tn == "InstMatmult":
                    if inst.ldweights:
                        last_ldw = str(inst.ins[1])
            if dead:
                ds = set(map(id, dead))
                blk.instructions[:] = [
                    i for i in blk.instructions if id(i) not in ds
                ]
            removed += len(dead)
    return removed


def _build(n_cap: int, reps: int = 1):
    """Per-core program: outT[o, n] = (xT.T @ WT).T, bf16 io.

    W-stationary schedule: the stationary operand is a [128,128] block of
    WT (64 blocks total), each loaded once and streamed against all n_cap
    rows of x (the moving operand), so PE streaming is 64*n_cap cycles --
    it scales with the actual row count instead of rounding up to full
    128-row tiles -- and the serialized LDWEIGHTS tax is 64 loads instead
    of one per matmul (the Tile scheduler's redundant reloads are stripped
    by _dedup_ldweights). Accumulation runs over k in PSUM per (o-block,
    row-chunk); evacuation is a single fp32->bf16 copy alternating between
    ACT and DVE. No bias on device: the host adds the row-independent
    summed bias during scatter.

    reps > 1 repeats the whole body (including all DMAs) back-to-back for
    benchmarking: wall(T) - wall(1) isolates per-execution device time."""
    assert n_cap % 32 == 0
    chunks = _chunks(n_cap)
    nc = bacc.Bacc("TRN2", target_bir_lowering=False, debug=False, num_devices=8, num_swdge_queues=4)
    xT = nc.dram_tensor("xT", [D, n_cap], mybir.dt.bfloat16, kind="ExternalInput").ap()
    WT = nc.dram_tensor("WT", [D, D], mybir.dt.bfloat16, kind="ExternalInput").ap()
    out = nc.dram_tensor("outT", [D, n_cap], mybir.dt.bfloat16, kind="ExternalOutput").ap()

    with tile.TileContext(nc) as tc, ExitStack() as ctx:
        w_pool = ctx.enter_context(tc.tile_pool(name="w", bufs=2))
        x_pool = ctx.enter_context(tc.tile_pool(name="x", bufs=2))
        o_pool = ctx.enter_context(tc.tile_pool(name="o", bufs=3))
        ps_pool = ctx.enter_context(tc.tile_pool(name="ps", bufs=2, space="PSUM"))

        xh = n_cap // 64 * 32  # x column split point (row dim), 32-aligned

        for _rep in range(reps):
            w_tiles, x_tiles = [], []
            for k in range(KT):
                w_tiles.append(
                    w_pool.tile([P, D], mybir.dt.bfloat16, name=f"wt{k}", tag=f"wt{k}")
                )
                x_tiles.append(
                    x_pool.tile([P, n_cap], mybir.dt.bfloat16, name=f"xt{k}", tag=f"xt{k}")
                )
            # W lands h0-halves first: o-passes 0..3 only touch them.
            for lo in (0, HALF):
                for k in range(KT):
                    nc.sync.dma_start(
                        w_tiles[k][:, lo : lo + HALF], WT[k * P : (k + 1) * P, lo : lo + HALF]
                    )
            for k in range(KT):
                nc.gpsimd.dma_start(x_tiles[k][:, 0:xh], xT[k * P : (k + 1) * P, 0:xh])
                nc.gpsimd.dma_start(x_tiles[k][:, xh:n_cap], xT[k * P : (k + 1) * P, xh:n_cap])

            for o in range(KT):
                ps_t = [
                    ps_pool.tile([P, sz], mybir.dt.float32, name=f"ps{o}_{cs}", tag=f"ps{ci}")
                    for ci, (cs, sz) in enumerate(chunks)
                ]
                for k in range(KT):
                    for ci, (cs, sz) in enumerate(chunks):
                        nc.tensor.matmul(
                            ps_t[ci][:, :],
                            w_tiles[k][:, o * P : (o + 1) * P],
                            x_tiles[k][:, cs : cs + sz],
                            start=(k == 0),
                            stop=(k == KT - 1),
                            skip_group_check=True,
                        )
                for ci, (cs, sz) in enumerate(chunks):
                    ob = o_pool.tile(
                        [P, sz], mybir.dt.bfloat16, name=f"ob{o}_{cs}", tag=f"ob{ci}"
                    )
                    if (o + ci) % 2 == 0:
                        nc.scalar.copy(ob[:, :], ps_t[ci][:, :])
                    else:
                        nc.vector.tensor_copy(ob[:, :], ps_t[ci][:, :])
                    out_eng = nc.scalar if (o + ci) % 2 == 0 else nc.gpsimd
                    out_eng.dma_start(out[o * P : (o + 1) * P, cs : cs + sz], ob[:, :])

    n_removed = _dedup_ldweights(nc)
    nc.compile()
    _check_noload_pairs(nc)
    nc._ldw_removed = n_removed
    return nc


def _check_noload_pairs(nc):
    """Every ldweights=False matmul must execute (in PE stream order) with
    the stationary operand most recently loaded into its row group — by a
    standalone InstLdweights or a self-loading matmul. Scheduling is
    deterministic at build time, so passing here guarantees correctness on
    device."""
    loaded: dict = {}
    for fn in nc.m.functions:
        for blk in fn.blocks:
            for inst in blk.instructions:
                tn = type(inst).__name__
                if tn == "InstLdweights":
                    rg = (inst.tile_position or (0, 0))[0]
                    loaded[rg] = str(inst.ins[0])
                elif tn == "InstMatmult":
                    rg = (inst.tile_position or (0, 0))[0]
                    if inst.ldweights is False:
                        assert loaded.get(rg) == str(inst.ins[1]), (
                            f"no-load matmul stationary mismatch in rg {rg}:\n"
                            f"loaded: {loaded.get(rg)}\nthis: {inst.ins[1]}"
                        )
                    else:
                        loaded[rg] = str(inst.ins[1])


class _Runner:
    """Caches the compiled NEFF + jitted shard_map executable for one n_cap."""

    def __init__(self, n_cap: int, reps: int = 1):
        self.n_cap = n_cap
        self.nc = _build(n_cap, reps)
        _b2j.install_neuronx_cc_hook()

        assert self.nc.dbg_addr is None
        partition_name = (
            self.nc.partition_id_tensor.name if self.nc.partition_id_tensor else None
        )

        in_names, out_names, out_avals = [], [], []
        for alloc in self.nc.m.functions[0].allocations:
            if not isinstance(alloc, mybir.MemoryLocationSet):
                continue
            name = alloc.memorylocations[0].name
            if alloc.kind == "ExternalInput":
                if name != partition_name:
                    in_names.append(name)
            elif alloc.kind == "ExternalOutput":
                out_names.append(name)
                out_avals.append(
                    jax.core.ShapedArray(
                        tuple(alloc.tensor_shape), mybir.dt.np(alloc.dtype)
                    )
                )
        self.in_names = in_names
        self.out_names = out_names
        self.out_avals = out_avals
        self.n_params = len(in_names)
        self.n_outs = len(out_names)
        all_in_names = tuple(in_names + out_names)
        if partition_name is not None:
            all_in_names = all_in_names + (partition_name,)

        nc = self.nc

        def _bind(*args):
            operands = list(args)
            if partition_name is not None:
                operands.append(_b2j.partition_id_tensor())
            return tuple(
                _b2j._bass_exec_p.bind(
                    *operands,
                    out_avals=tuple(out_avals),
                    in_names=all_in_names,
                    out_names=tuple(out_names),
                    lowering_input_output_aliases=(),
                    sim_require_finite=True,
                    sim_require_nnan=True,
                    nc=nc,
                )
            )

        self._bind = _bind
        self.devices = jax.devices("neuron")[:C]
        self.mesh = Mesh(np.asarray(self.devices), ("core",))
        spec_in = (PartitionSpec("core"),) * (self.n_params + self.n_outs)
        spec_out = (PartitionSpec("core"),) * self.n_outs
        self._spec_in, self._spec_out = spec_in, spec_out
        self._exec = jax.jit(
            shard_map(
                _bind,
                mesh=self.mesh,
                in_specs=spec_in,
                out_specs=spec_out,
                check_rep=False,
            ),
            donate_argnums=tuple(range(self.n_params, self.n_params + self.n_outs)),
            keep_unused=True,
        )

    def make_exec_nodonate(self):
        """Jitted executable that does not donate its output-init operands,
        so pre-staged device args can be reused across timing reps."""
        return jax.jit(
            shard_map(
                self._bind,
                mesh=self.mesh,
                in_specs=self._spec_in,
                out_specs=self._spec_out,
                check_rep=False,
            ),
            keep_unused=True,
        )

    def concat_inputs(self, in_maps):
        return [
            np.concatenate([np.asarray(m[name]) for m in in_maps], axis=0)
            for name in self.in_names
        ]

    def zero_outs(self):
        return [
            np.zeros((C * a.shape[0], *a.shape[1:]), a.dtype) for a in self.out_avals
        ]

    def run(self, in_maps):
        out_arrs = self._exec(*self.concat_inputs(in_maps), *self.zero_outs())
        return [
            {
                name: np.asarray(out_arrs[i]).reshape(C, *self.out_avals[i].shape)[c]
                for i, name in enumerate(self.out_names)
            }
            for c in range(C)
        ]


def _get(n_cap: int, reps: int = 1) -> _Runner:
    key = (n_cap, reps)
    if key not in _cache:
        _cache[key] = _Runner(n_cap, reps)
    return _cache[key]


def _prep(x, condition_ids, W, b):
    x = np.asarray(x, dtype=np.float32)
    cond = np.asarray(condition_ids).astype(np.int64)
    W = np.asarray(W, dtype=np.float32)
    b = np.asarray(b, dtype=np.float32)

    bias_sum = b.sum(axis=0, dtype=np.float32)

    rows = [np.nonzero(cond == c)[0] for c in range(C)]
    n_max = max(len(r) for r in rows)
    n_cap = max(32, -(-n_max // 32) * 32)

    in_maps = []
    for c in range(C):
        r = rows[c]
        xg = np.zeros((n_cap, D), BF16)
        xg[: len(r)] = x[r].astype(BF16)
        in_maps.append(
            {
                "xT": np.ascontiguousarray(xg.T),
                "WT": np.ascontiguousarray(W[c].T.astype(BF16)),
            }
        )
    return rows, n_cap, in_maps, bias_sum


def _run(x, condition_ids, W, b, trace=False):
    rows, n_cap, in_maps, bias_sum = _prep(x, condition_ids, W, b)
    runner = _get(n_cap)
    results = runner.run(in_maps)

    out = np.empty((B, D), np.float32)
    for c in range(C):
        r = rows[c]
        out[r] = results[c]["outT"][:, : len(r)].T.astype(np.float32) + bias_sum
    return out, runner


def kernel(x, condition_ids, W, b):
    out, _ = _run(x, condition_ids, W, b)
    return out


# revision 13
# speedup vs baseline: 1.9472x; 1.4535x over previous
"""Conditional (per-row expert) linear layer for Trainium2, 8 NeuronCores.

Math: out[i] = W[c_i] @ x[i] + sum_c b[c]    (x: [B,D], W: [C,D,D], b: [C,D])

Strategy: expert-parallel. Core c handles exactly the rows with
condition_ids == c (gathered on host, padded to a common capacity so the
SPMD NEFF has static shapes). Each core runs one [n_cap, D] @ [D, D] GEMM
with bf16 operands (fp32 PSUM accumulation), the host scatters rows back
and adds the (row-independent) summed bias in fp32. bf16 halves HBM
traffic vs fp32 (x 2.25MB + W 2MB + out 2.25MB per core) and enables the
PE's fast-weight-load path, so the kernel is bound by PE streaming:
n_cap*D*D/128^2 cycles @ 2.4GHz (~30.7us at n_cap=1152) plus per-k-tile
weight loads.
"""

import sys
from contextlib import ExitStack

import numpy as np
import ml_dtypes

try:
    import concourse.bass as bass  # noqa: F401
except ImportError:  # pragma: no cover
    sys.path.insert(0, "/opt/trn_rl_repo")

import jax
from jax.experimental.shard_map import shard_map
from jax.sharding import Mesh, PartitionSpec

import concourse.mybir as mybir
import concourse.tile as tile
from concourse import bacc
from concourse import bass2jax as _b2j

B, D, C = 8192, 1024, 8
P = 128  # partitions
KT = D // P  # k-tiles along the contraction dim
HALF = 512  # PSUM half-bank free size (fp32)
BF16 = ml_dtypes.bfloat16

_cache: dict[tuple, "_Runner"] = {}
DMA_ONCE = False  # diag: hoist input DMA out of the rep loop


def _chunks(n_cap: int):
    """Split the row range into near-equal chunks of <=512 (PSUM bank)."""
    n = -(-n_cap // HALF)
    base = n_cap // n
    sizes = [base + (1 if i < n_cap - base * n else 0) for i in range(n)]
    out, s = [], 0
    for sz in sizes:
        out.append((s, sz))
        s += sz
    assert s == n_cap
    return out


def _dedup_ldweights(nc):
    """The Tile scheduler pairs every InstMatmult with its own
    InstLdweights even when consecutive matmuls share the stationary
    operand. Drop the redundant reloads (PE stream order, same stationary
    AP, no sync side effects) — each costs ~53ns of serial PE time.
    Runs before nc.compile() so the wait-migration passes see the final
    stream; _check_noload_pairs re-verifies after compile."""
    removed = 0
    for fn in nc.m.functions:
        for blk in fn.blocks:
            last_ldw = None
            dead = []
            for inst in blk.instructions:
                tn = type(inst).__name__
                if tn == "InstLdweights":
                    cur = str(inst.ins[0])
                    si = inst.sync_info
                    clean = si is None or (not si.on_wait and not si.on_update)
                    if last_ldw == cur and clean:
                        dead.append(inst)
                    else:
                        last_ldw = cur
                elif tn == "InstMatmult":
                    if inst.ldweights:
                        last_ldw = str(inst.ins[1])
            if dead:
                ds = set(map(id, dead))
                blk.instructions[:] = [
                    i for i in blk.instructions if id(i) not in ds
                ]
            removed += len(dead)
    return removed


def _build(n_cap: int, reps: int = 1):
    """Per-core program: outT[o, n] = (xT.T @ WT).T, bf16 io.

    W-stationary schedule: the stationary operand is a [128,128] block of
    WT (64 blocks total), each loaded once and streamed against all n_cap
    rows of x (the moving operand), so PE streaming is 64*n_cap cycles --
    it scales with the actual row count instead of rounding up to full
    128-row tiles -- and the serialized LDWEIGHTS tax is 64 loads instead
    of one per matmul (the Tile scheduler's redundant reloads are stripped
    by _dedup_ldweights). Accumulation runs over k in PSUM per (o-block,
    row-chunk); evacuation is a single fp32->bf16 copy alternating between
    ACT and DVE. No bias on device: the host adds the row-independent
    summed bias during scatter.

    reps > 1 repeats the whole body (including all DMAs) back-to-back for
    benchmarking: wall(T) - wall(1) isolates per-execution device time."""
    assert n_cap % 16 == 0
    chunks = _chunks(n_cap)
    nc = bacc.Bacc("TRN2", target_bir_lowering=False, debug=False, num_devices=8, num_swdge_queues=4)
    xT = nc.dram_tensor("xT", [D, n_cap], mybir.dt.bfloat16, kind="ExternalInput").ap()
    WT = nc.dram_tensor("WT", [D, D], mybir.dt.bfloat16, kind="ExternalInput").ap()
    out = nc.dram_tensor("outT", [D, n_cap], mybir.dt.bfloat16, kind="ExternalOutput").ap()

    with tile.TileContext(nc) as tc, ExitStack() as ctx:
        w_pool = ctx.enter_context(tc.tile_pool(name="w", bufs=2))
        x_pool = ctx.enter_context(tc.tile_pool(name="x", bufs=2))
        o_pool = ctx.enter_context(tc.tile_pool(name="o", bufs=3))
        ps_pool = ctx.enter_context(tc.tile_pool(name="ps", bufs=2, space="PSUM"))

        xh = n_cap // 32 * 16  # x column split point (row dim), 16-aligned

        for _rep in range(reps):
            if _rep == 0 or not DMA_ONCE:
                w_tiles, x_tiles = [], []
                for k in range(KT):
                    w_tiles.append(
                        w_pool.tile([P, D], mybir.dt.bfloat16, name=f"wt{k}", tag=f"wt{k}")
                    )
                    x_tiles.append(
                        x_pool.tile([P, n_cap], mybir.dt.bfloat16, name=f"xt{k}", tag=f"xt{k}")
                    )
                # W lands h0-halves first: o-passes 0..3 only touch them.
                for lo in (0, HALF):
                    for k in range(KT):
                        nc.sync.dma_start(
                            w_tiles[k][:, lo : lo + HALF], WT[k * P : (k + 1) * P, lo : lo + HALF]
                        )
                for k in range(KT):
                    nc.gpsimd.dma_start(x_tiles[k][:, 0:xh], xT[k * P : (k + 1) * P, 0:xh])
                    nc.gpsimd.dma_start(x_tiles[k][:, xh:n_cap], xT[k * P : (k + 1) * P, xh:n_cap])

            for o in range(KT):
                ps_t = [
                    ps_pool.tile([P, sz], mybir.dt.float32, name=f"ps{o}_{cs}", tag=f"ps{ci}")
                    for ci, (cs, sz) in enumerate(chunks)
                ]
                for k in range(KT):
                    for ci, (cs, sz) in enumerate(chunks):
                        nc.tensor.matmul(
                            ps_t[ci][:, :],
                            w_tiles[k][:, o * P : (o + 1) * P],
                            x_tiles[k][:, cs : cs + sz],
                            start=(k == 0),
                            stop=(k == KT - 1),
                            skip_group_check=True,
                        )
                for ci, (cs, sz) in enumerate(chunks):
                    ob = o_pool.tile(
                        [P, sz], mybir.dt.bfloat16, name=f"ob{o}_{cs}", tag=f"ob{ci}"
                    )
                    if (o + ci) % 2 == 0:
                        nc.scalar.copy(ob[:, :], ps_t[ci][:, :])
                    else:
                        nc.vector.tensor_copy(ob[:, :], ps_t[ci][:, :])
                    out_eng = nc.scalar if (o + ci) % 2 == 0 else nc.gpsimd
                    out_eng.dma_start(out[o * P : (o + 1) * P, cs : cs + sz], ob[:, :])

    n_removed = _dedup_ldweights(nc)
    nc.compile()
    _check_noload_pairs(nc)
    nc._ldw_removed = n_removed
    return nc


def _check_noload_pairs(nc):
    """Every ldweights=False matmul must execute (in PE stream order) with
    the stationary operand most recently loaded into its row group — by a
    standalone InstLdweights or a self-loading matmul. Scheduling is
    deterministic at build time, so passing here guarantees correctness on
    device."""
    loaded: dict = {}
    for fn in nc.m.functions:
        for blk in fn.blocks:
            for inst in blk.instructions:
                tn = type(inst).__name__
                if tn == "InstLdweights":
                    rg = (inst.tile_position or (0, 0))[0]
                    loaded[rg] = str(inst.ins[0])
                elif tn == "InstMatmult":
                    rg = (inst.tile_position or (0, 0))[0]
                    if inst.ldweights is False:
                        assert loaded.get(rg) == str(inst.ins[1]), (
                            f"no-load matmul stationary mismatch in rg {rg}:\n"
                            f"loaded: {loaded.get(rg)}\nthis: {inst.ins[1]}"
                        )
                    else:
                        loaded[rg] = str(inst.ins[1])


class _Runner:
    """Caches the compiled NEFF + jitted shard_map executable for one n_cap."""

    def __init__(self, n_cap: int, reps: int = 1):
        self.n_cap = n_cap
        self.nc = _build(n_cap, reps)
        _b2j.install_neuronx_cc_hook()

        assert self.nc.dbg_addr is None
        partition_name = (
            self.nc.partition_id_tensor.name if self.nc.partition_id_tensor else None
        )

        in_names, out_names, out_avals = [], [], []
        for alloc in self.nc.m.functions[0].allocations:
            if not isinstance(alloc, mybir.MemoryLocationSet):
                continue
            name = alloc.memorylocations[0].name
            if alloc.kind == "ExternalInput":
                if name != partition_name:
                    in_names.append(name)
            elif alloc.kind == "ExternalOutput":
                out_names.append(name)
                out_avals.append(
                    jax.core.ShapedArray(
                        tuple(alloc.tensor_shape), mybir.dt.np(alloc.dtype)
                    )
                )
        self.in_names = in_names
        self.out_names = out_names
        self.out_avals = out_avals
        self.n_params = len(in_names)
        self.n_outs = len(out_names)
        all_in_names = tuple(in_names + out_names)
        if partition_name is not None:
            all_in_names = all_in_names + (partition_name,)

        nc = self.nc

        def _bind(*args):
            operands = list(args)
            if partition_name is not None:
                operands.append(_b2j.partition_id_tensor())
            return tuple(
                _b2j._bass_exec_p.bind(
                    *operands,
                    out_avals=tuple(out_avals),
                    in_names=all_in_names,
                    out_names=tuple(out_names),
                    lowering_input_output_aliases=(),
                    sim_require_finite=True,
                    sim_require_nnan=True,
                    nc=nc,
                )
            )

        self._bind = _bind
        self.devices = jax.devices("neuron")[:C]
        self.mesh = Mesh(np.asarray(self.devices), ("core",))
        spec_in = (PartitionSpec("core"),) * (self.n_params + self.n_outs)
        spec_out = (PartitionSpec("core"),) * self.n_outs
        self._spec_in, self._spec_out = spec_in, spec_out
        self._exec = jax.jit(
            shard_map(
                _bind,
                mesh=self.mesh,
                in_specs=spec_in,
                out_specs=spec_out,
                check_rep=False,
            ),
            donate_argnums=tuple(range(self.n_params, self.n_params + self.n_outs)),
            keep_unused=True,
        )

    def make_exec_nodonate(self):
        """Jitted executable that does not donate its output-init operands,
        so pre-staged device args can be reused across timing reps."""
        return jax.jit(
            shard_map(
                self._bind,
                mesh=self.mesh,
                in_specs=self._spec_in,
                out_specs=self._spec_out,
                check_rep=False,
            ),
            keep_unused=True,
        )

    def concat_inputs(self, in_maps):
        return [
            np.concatenate([np.asarray(m[name]) for m in in_maps], axis=0)
            for name in self.in_names
        ]

    def zero_outs(self):
        return [
            np.zeros((C * a.shape[0], *a.shape[1:]), a.dtype) for a in self.out_avals
        ]

    def run(self, in_maps):
        out_arrs = self._exec(*self.concat_inputs(in_maps), *self.zero_outs())
        return [
            {
                name: np.asarray(out_arrs[i]).reshape(C, *self.out_avals[i].shape)[c]
                for i, name in enumerate(self.out_names)
            }
            for c in range(C)
        ]


def _get(n_cap: int, reps: int = 1) -> _Runner:
    key = (n_cap, reps, DMA_ONCE)
    if key not in _cache:
        _cache[key] = _Runner(n_cap, reps)
    return _cache[key]


def _prep(x, condition_ids, W, b):
    x = np.asarray(x, dtype=np.float32)
    cond = np.asarray(condition_ids).astype(np.int64)
    W = np.asarray(W, dtype=np.float32)
    b = np.asarray(b, dtype=np.float32)

    bias_sum = b.sum(axis=0, dtype=np.float32)

    rows = [np.nonzero(cond == c)[0] for c in range(C)]
    n_max = max(len(r) for r in rows)
    n_cap = max(32, -(-n_max // 16) * 16)

    in_maps = []
    for c in range(C):
        r = rows[c]
        xg = np.zeros((n_cap, D), BF16)
        xg[: len(r)] = x[r].astype(BF16)
        in_maps.append(
            {
                "xT": np.ascontiguousarray(xg.T),
                "WT": np.ascontiguousarray(W[c].T.astype(BF16)),
            }
        )
    return rows, n_cap, in_maps, bias_sum


def _run(x, condition_ids, W, b, trace=False):
    rows, n_cap, in_maps, bias_sum = _prep(x, condition_ids, W, b)
    runner = _get(n_cap)
    results = runner.run(in_maps)

    out = np.empty((B, D), np.float32)
    for c in range(C):
        r = rows[c]
        out[r] = results[c]["outT"][:, : len(r)].T.astype(np.float32) + bias_sum
    return out, runner


def kernel(x, condition_ids, W, b):
    out, _ = _run(x, condition_ids, W, b)
    return out
